# revision 20
# baseline (speedup 1.0000x reference)
"""Trainium2 Bass kernel for a 3-layer difflogic network (nn_Net_48610439856713).

Math: each layer o computes softmax(w[o])·ops16(a, b) with a = h[:, ia[o]],
b = h[:, ib[o]].  The 16 relaxed logic gates are all affine in {1, a, b, ab},
so the layer reduces to  h' = C0 + C1·a + C2·b + C3·a·b  with 4 per-neuron
coefficients derived on-device from softmax(w).

Sharding: 2 batch groups x 4 neuron shards over the 8 cores.  Core c handles
batch rows [(c//4)*256, ...) and neuron shard c%4 of every layer.  Activations
are bf16 in a transposed packed layout h^T[neuron, batch]; each layer's shard
outputs are exchanged with a 4-rank AllGather so every core holds the full
previous layer as its gather source.  Gathers use the SWDGE dma_gather
primitive (cost ~8ns/index of Q7 descriptor generation, the kernel's
bottleneck — which is why indices per core are minimized via neuron sharding).

Host-side bookkeeping is integer/layout only: slot permutations, index
relabeling through the packed layout, int16 index wrapping, weight-row
packing.  All float arithmetic (softmax, combine, sums) runs on device.
"""

import os
import numpy as np

P = 128
B = 512
BG = 2                  # batch groups
SH = 4                  # neuron shards
BC = B // BG            # 256 batch per core
IN = 193
NGROUP = 3
TAU = 100.0
N_CORES = 8

# layers 1/2: 16000 real neurons -> 4096 slots/shard (96 pads each)
NJ12 = 32               # j-columns per shard
REAL12 = 4000           # real neurons per shard
NS12 = NJ12 * P         # 4096 slots per shard
# layer 3: 15999 real -> 33 j-cols/shard; group g owns local j in [11g, 11g+11)
NJ3 = 33
JPG = 11                # j-cols per group per shard
NS3 = NJ3 * P           # 4224 slots per shard
SPG = 15999 // NGROUP   # 5333 real slots per group

_CACHE = {}


def _build_nc():
    import concourse.bacc as bacc
    import concourse.tile as tile
    import concourse.mybir as mybir

    f32 = mybir.dt.float32
    bf16 = mybir.dt.bfloat16
    fp8 = mybir.dt.float8e4
    i16 = mybir.dt.int16
    Alu = mybir.AluOpType
    Act = mybir.ActivationFunctionType
    Ax = mybir.AxisListType

    nc = bacc.Bacc(
        "TRN2", target_bir_lowering=False, debug=False, num_devices=N_CORES,
        num_swdge_queues=4,
    )

    # ---- I/O ----
    xT = nc.dram_tensor("xT", [IN, BC], bf16, kind="ExternalInput")
    wps = [
        nc.dram_tensor("w1p", [P, NJ12 * 16], f32, kind="ExternalInput"),
        nc.dram_tensor("w2p", [P, NJ12 * 16], f32, kind="ExternalInput"),
        nc.dram_tensor("w3p", [P, NJ3 * 16], f32, kind="ExternalInput"),
    ]
    idxs = []
    for l, ns in ((1, NS12), (2, NS12), (3, NS3)):
        # combined a+b index stream, chunk-interleaved: [a-chunk0 b-chunk0 ...]
        idxs.append(
            nc.dram_tensor(f"i{l}", [P, 2 * ns // 16], i16, kind="ExternalInput")
        )
    out_d = nc.dram_tensor("out", [1, NGROUP * BC], f32, kind="ExternalOutput")

    # collective buffers (h exchange, NCH j-chunks pipelined) and partial-sum
    # exchange.  g layout is chunk-major: row r = k*SH*P + s*P + p, unit
    # r*JCH + (j % JCH)  with JCH = NJ12//NCH j-cols per chunk.
    NCH_ = NCH
    JCH_ = JCH
    cins = [
        [
            nc.dram_tensor(f"cin{l}_{k}", [P, JCH * BC], fp8, kind="Internal")
            for k in range(NCH)
        ]
        for l in (1, 2)
    ]
    gs_ = [
        nc.dram_tensor("g1", [NCH * SH * P, JCH * BC], fp8, kind="Internal"),
        nc.dram_tensor("g2", [NCH * SH * P, JCH * BC], fp8, kind="Internal"),
    ]
    # warm-up collective: absorbs first-collective firmware latency while the
    # layer-1 gathers run.
    win = nc.dram_tensor("win", [P, 16], f32, kind="Internal")
    warm = nc.dram_tensor("warm", [SH * P, 16], f32, kind="Internal")
    pin = nc.dram_tensor("pin", [1, NGROUP * BC], f32, kind="Internal")
    pall = nc.dram_tensor("pall", [SH, NGROUP * BC], f32, kind="Internal")

    shard_groups = [[0, 1, 2, 3], [4, 5, 6, 7]]

    with tile.TileContext(nc) as tc:
        with (
            tc.tile_pool(name="big", bufs=1) as big,
            tc.tile_pool(name="prep", bufs=2) as prep,
            tc.tile_pool(name="small", bufs=2) as small,
            tc.tile_pool(name="psum", bufs=1, space="PSUM") as psum,
        ):
            layers = [
                (NJ12, NS12, bf16, bf16, fp8, xT[:], idxs[0], wps[0], cins[0], gs_[0]),
                (
                    NJ12, NS12, fp8, bf16, fp8,
                    gs_[0][:].rearrange("r (j b) -> (r j) b", b=BC),
                    idxs[1], wps[1], cins[1], gs_[1],
                ),
                (
                    NJ3, NS3, fp8, bf16, bf16,
                    gs_[1][:].rearrange("r (j b) -> (r j) b", b=BC),
                    idxs[2], wps[2], None, None,
                ),
            ]  # per layer: (NJ, NS, gather-dtype, tmp-dtype, h-dtype, src, ...)

            wsb = prep.tile([P, 16], f32, tag="wsb")
            nc.vector.memset(wsb[:], 0.0)
            nc.sync.dma_start(win[:], wsb[:])
            nc.gpsimd.collective_compute(
                "AllGather", Alu.bypass, replica_groups=shard_groups,
                ins=[win[:]], outs=[warm[:]],
            )

            h_final = None
            for li, (NJ, NS, gdt, tdt, hdt, src, iad, wp, cin, gout) in enumerate(
                layers
            ):
                last = li == 2
                # ---- coefficient prep: C0..C3 [P, NJ] f32 ----
                wt = prep.tile([P, NJ * 16], f32, tag="wt")
                nc.sync.dma_start(wt[:], wp[:])
                e = prep.tile([P, NJ * 16], f32, tag="e")
                nc.scalar.activation(e[:], wt[:], Act.Exp)
                e3 = e[:].rearrange("p (j g) -> p j g", g=16)
                e4 = e[:].rearrange("p (j h q) -> p j h q", h=4, q=4)

                ssum = small.tile([P, NJ], f32, tag="ssum")
                nc.vector.reduce_sum(ssum[:], e3, axis=Ax.X)
                r = small.tile([P, NJ], f32, tag="r")
                nc.vector.reciprocal(r[:], ssum[:])

                c0 = small.tile([P, NJ], f32, tag="c0")
                c1 = small.tile([P, NJ], f32, tag="c1")
                c2 = small.tile([P, NJ], f32, tag="c2")
                c3 = small.tile([P, NJ], f32, tag="c3")

                nc.vector.reduce_sum(c0[:], e4[:, :, 2:4, :], axis=Ax.XY)
                t1 = small.tile([P, NJ], f32, tag="t1")
                t2 = small.tile([P, NJ], f32, tag="t2")
                nc.vector.reduce_sum(t1[:], e4[:, :, 0:2, 2:4], axis=Ax.XY)
                nc.vector.reduce_sum(t2[:], e4[:, :, 2:4, 0:2], axis=Ax.XY)
                nc.vector.tensor_sub(c1[:], t1[:], t2[:])
                t3 = small.tile([P, NJ], f32, tag="t3")
                t4 = small.tile([P, NJ], f32, tag="t4")
                nc.vector.reduce_sum(t3[:], e4[:, :, 1, :], axis=Ax.X)
                nc.vector.reduce_sum(t4[:], e4[:, :, 2, :], axis=Ax.X)
                nc.vector.tensor_sub(c2[:], t3[:], t4[:])
                f = small.tile([P, NJ, 7], f32, tag="f")
                nc.vector.tensor_sub(f[:], e3[:, :, 1:8], e3[:, :, 14:7:-1])
                u1 = small.tile([P, NJ], f32, tag="u1")
                u2 = small.tile([P, NJ], f32, tag="u2")
                nc.vector.tensor_sub(u1[:], f[:, :, 0], f[:, :, 1])
                nc.vector.tensor_add(u2[:], f[:, :, 3], f[:, :, 6])
                nc.vector.tensor_sub(u1[:], u1[:], u2[:])
                nc.vector.scalar_tensor_tensor(
                    c3[:], f[:, :, 5], -2.0, u1[:], op0=Alu.mult, op1=Alu.add
                )
                for ck in (c0, c1, c2, c3):
                    nc.vector.tensor_mul(ck[:], ck[:], r[:])

                # ---- idx load ----
                iab = prep.tile([P, 2 * NS // 16], i16, tag="iab")
                nc.sync.dma_start(iab[:], iad[:])

                # ---- chunked gathers + combine ----
                h = big.tile([P, NJ * BC], hdt, tag="h")
                h3 = h[:].rearrange("p (j b) -> p j b", b=BC)
                if last:
                    # group-aligned chunks so GroupSum reduces fire per chunk
                    chunks = [(0, 11), (11, 22), (22, NJ)]
                else:
                    chunks = [(k * JCH, (k + 1) * JCH) for k in range(NCH)]
                for ci, (j0, j1) in enumerate(chunks):
                    cw = j1 - j0
                    ab = big.tile([P, 2 * cw, BC], gdt, tag=f"ab{ci % 2}")
                    nsc = 2 * cw * P
                    nc.gpsimd.dma_gather(
                        ab[:], src, iab[:, 2 * j0 * 8 : 2 * j1 * 8], nsc, nsc, BC,
                        single_packet=False, queue_num=ci % 4,
                    )
                    # h = (C1*a + C0) + (C3*a + C2)*b: u,v on Scalar; w,h on
                    # Vector — balances the two engines per j-column.
                    uv = big.tile([P, 2 * cw, BC], tdt, tag=f"uv{ci % 2}")
                    for j in range(j0, j1):
                        jl = j - j0
                        aj = ab[:, jl]
                        bj = ab[:, cw + jl]
                        nc.scalar.activation(
                            uv[:, jl], aj, Act.Identity,
                            bias=c0[:, j : j + 1], scale=c1[:, j : j + 1],
                        )
                        nc.scalar.activation(
                            uv[:, cw + jl], aj, Act.Identity,
                            bias=c2[:, j : j + 1], scale=c3[:, j : j + 1],
                        )
                        nc.vector.tensor_mul(uv[:, cw + jl], uv[:, cw + jl], bj)
                        nc.vector.tensor_add(h3[:, j], uv[:, cw + jl], uv[:, jl])

                    if not last:
                        # ship this chunk as soon as it's combined
                        nc.sync.dma_start(
                            cin[ci][:], h[:, j0 * BC : j1 * BC]
                        )
                        nc.gpsimd.collective_compute(
                            "AllGather", Alu.bypass, replica_groups=shard_groups,
                            ins=[cin[ci][:]],
                            outs=[gout[ci * SH * P : (ci + 1) * SH * P, :]],
                        )
                if last:
                    h_final = h

            # ---- GroupSum: per-shard partials, then cross-shard AllGather+sum ----
            gs = prep.tile([P, NGROUP * BC], f32, tag="gs")
            for g in range(NGROUP):
                sl = h_final[:, g * JPG * BC : (g + 1) * JPG * BC].rearrange(
                    "p (j b) -> p b j", b=BC
                )
                nc.vector.reduce_sum(gs[:, g * BC : (g + 1) * BC], sl, axis=Ax.X)
            ones = prep.tile([P, 1], f32, tag="ones")
            nc.vector.memset(ones[:], 1.0)
            psc = prep.tile([1, NGROUP * BC], f32, tag="psc")
            HW = NGROUP * BC // 2
            for k in range(2):
                ps = psum.tile([1, HW], f32, tag=f"ps{k}")
                nc.tensor.matmul(
                    ps[:], ones[:], gs[:, k * HW : (k + 1) * HW],
                    start=True, stop=True,
                )
                nc.scalar.copy(psc[:, k * HW : (k + 1) * HW], ps[:])
            nc.sync.dma_start(pin[:], psc[:])
            nc.gpsimd.collective_compute(
                "AllGather", Alu.bypass, replica_groups=shard_groups,
                ins=[pin[:]], outs=[pall[:]],
            )
            pall_sb = prep.tile([SH, NGROUP * BC], f32, tag="pall_sb")
            nc.sync.dma_start(pall_sb[:], pall[:])
            ones4 = prep.tile([SH, 1], f32, tag="ones4")
            nc.vector.memset(ones4[:], 1.0)
            osb = prep.tile([1, NGROUP * BC], f32, tag="osb")
            for k in range(2):
                ps2 = psum.tile([1, HW], f32, tag=f"ps2{k}")
                nc.tensor.matmul(
                    ps2[:], ones4[:], pall_sb[:, k * HW : (k + 1) * HW],
                    start=True, stop=True,
                )
                nc.scalar.mul(osb[:, k * HW : (k + 1) * HW], ps2[:], 1.0 / TAU)
            # consume the warm-up collective's (all-zero) output so DCE keeps it
            wsb2 = prep.tile([1, 16], f32, tag="wsb2")
            nc.sync.dma_start(wsb2[:], warm[0:1, :])
            nc.vector.tensor_add(osb[:, :16], osb[:, :16], wsb2[:])
            nc.sync.dma_start(out_d[:], osb[:])

    nc.compile()
    return nc


def _wrap_idx(ii):
    w = ii.astype(np.int16).reshape(-1, 16).T
    return np.ascontiguousarray(np.tile(w, (8, 1)))


CHUNKS12 = [(0, 8), (8, 16), (16, 24), (24, 32)]
CHUNKS3 = [(0, 11), (11, 22), (22, 33)]


def _combine_idx(ia_eff, ib_eff, chunk_list):
    """Interleave a/b index streams per chunk: [a-chunk0, b-chunk0, a-chunk1, ...]"""
    parts = []
    for j0, j1 in chunk_list:
        parts.append(ia_eff[j0 * P : j1 * P])
        parts.append(ib_eff[j0 * P : j1 * P])
    return _wrap_idx(np.concatenate(parts))


def _pack_w(w_eff, nj):
    # local slot t = j*128 + p  ->  packed[p, j*16+g]
    return np.ascontiguousarray(
        w_eff.reshape(nj, P, 16).transpose(1, 0, 2).reshape(P, nj * 16)
    )


NCH = 4
JCH = NJ12 // NCH


def _src_unit12(i):
    """BC-row unit of layer-1/2 neuron i in the chunk-major AllGathered
    [NCH*SH*128, JCH*BC] layout: shard s = i//4000, local t = i - 4000s,
    p = t%128, j = t//128, chunk k = j//JCH; row = (k*SH+s)*128+p,
    unit = row*JCH + j%JCH."""
    s = i // REAL12
    t = i - s * REAL12
    p = t % P
    j = t // P
    k = j // JCH
    return ((k * SH + s) * P + p) * JCH + j % JCH


def _host_pack(inputs):
    x = np.asarray(inputs["x"], dtype=np.float32)
    w1 = np.asarray(inputs["w1"], dtype=np.float32)
    w2 = np.asarray(inputs["w2"], dtype=np.float32)
    w3 = np.asarray(inputs["w3"], dtype=np.float32)
    i1a = np.asarray(inputs["idx1a"]).astype(np.int64)
    i1b = np.asarray(inputs["idx1b"]).astype(np.int64)
    i2a = np.asarray(inputs["idx2a"]).astype(np.int64)
    i2b = np.asarray(inputs["idx2b"]).astype(np.int64)
    i3a = np.asarray(inputs["idx3a"]).astype(np.int64)
    i3b = np.asarray(inputs["idx3b"]).astype(np.int64)

    pad_row = np.full(16, -20.0, dtype=np.float32)
    pad_row[0] = 20.0  # softmax -> ~one-hot FALSE gate -> h = 0

    per_shard = [dict() for _ in range(SH)]
    # layers 1 and 2: shard s owns real neurons [s*4000, (s+1)*4000)
    for l, (w, ja, jb, srcf) in enumerate(
        (
            (w1, i1a, i1b, lambda i: i),
            (w2, i2a, i2b, _src_unit12),
        ),
        start=1,
    ):
        for s in range(SH):
            sel = slice(s * REAL12, (s + 1) * REAL12)
            w_eff = np.concatenate(
                [w[sel], np.tile(pad_row, (NS12 - REAL12, 1))], axis=0
            )
            ia_eff = np.zeros(NS12, dtype=np.int64)
            ib_eff = np.zeros(NS12, dtype=np.int64)
            ia_eff[:REAL12] = srcf(ja[sel])
            ib_eff[:REAL12] = srcf(jb[sel])
            per_shard[s][f"w{l}p"] = _pack_w(w_eff, NJ12)
            per_shard[s][f"i{l}"] = _combine_idx(ia_eff, ib_eff, CHUNKS12)

    # layer 3: group g's 5333 real neurons split over shards as
    # counts c_s = [1334, 1333, 1333, 1333]; within (s, g): local j in
    # [11g, 11g+11), rank m = (j-11g)*128 + p
    counts = np.array([1334, 1333, 1333, 1333])
    offs = np.concatenate([[0], np.cumsum(counts)[:-1]])
    u = np.arange(NS3)
    jj = u // P
    pp = u % P
    gg = jj // JPG
    m = (jj - gg * JPG) * P + pp
    for s in range(SH):
        real = m < counts[s]
        rid = gg * SPG + offs[s] + np.minimum(m, counts[s] - 1)
        w3_eff = w3[rid].copy()
        w3_eff[~real] = pad_row
        i3a_eff = np.where(real, _src_unit12(i3a[rid]), 0)
        i3b_eff = np.where(real, _src_unit12(i3b[rid]), 0)
        per_shard[s]["w3p"] = _pack_w(w3_eff, NJ3)
        per_shard[s]["i3"] = _combine_idx(i3a_eff, i3b_eff, CHUNKS3)

    import ml_dtypes

    in_maps = []
    for c in range(N_CORES):
        G, s = c // SH, c % SH
        m_ = dict(per_shard[s])
        m_["xT"] = np.ascontiguousarray(
            x[G * BC : (G + 1) * BC].T.astype(ml_dtypes.bfloat16)
        )
        in_maps.append(m_)
    return in_maps


LAST_RESULTS = None


def kernel(**inputs):
    global LAST_RESULTS
    from concourse.bass_utils import run_bass_kernel_spmd

    if "nc" not in _CACHE:
        _CACHE["nc"] = _build_nc()
    nc = _CACHE["nc"]

    in_maps = _host_pack(inputs)
    trace = bool(int(os.environ.get("KERNEL_TRACE", "0")))
    res = run_bass_kernel_spmd(
        nc, in_maps, core_ids=list(range(N_CORES)), trace=trace
    )
    LAST_RESULTS = res

    out = np.empty((B, NGROUP), dtype=np.float32)
    for g_ in range(BG):
        rc = res.results[g_ * SH]["out"].reshape(NGROUP, BC)
        out[g_ * BC : (g_ + 1) * BC, :] = rc.T
    return out



# revision 22
# speedup vs baseline: 1.0462x; 1.0462x over previous
"""Trainium2 Bass kernel for a 3-layer difflogic network (nn_Net_48610439856713).

Math: each layer o computes softmax(w[o])·ops16(a, b) with a = h[:, ia[o]],
b = h[:, ib[o]].  The 16 relaxed logic gates are all affine in {1, a, b, ab},
so the layer reduces to  h' = C0 + C1·a + C2·b + C3·a·b  with 4 per-neuron
coefficients derived on-device from softmax(w).

Sharding: 2 batch groups x 4 neuron shards over the 8 cores.  Core c handles
batch rows [(c//4)*256, ...) and neuron shard c%4 of every layer.  Activations
are bf16 in a transposed packed layout h^T[neuron, batch]; each layer's shard
outputs are exchanged with a 4-rank AllGather so every core holds the full
previous layer as its gather source.  Gathers use the SWDGE dma_gather
primitive (cost ~8ns/index of Q7 descriptor generation, the kernel's
bottleneck — which is why indices per core are minimized via neuron sharding).

Host-side bookkeeping is integer/layout only: slot permutations, index
relabeling through the packed layout, int16 index wrapping, weight-row
packing.  All float arithmetic (softmax, combine, sums) runs on device.
"""

import os
import numpy as np

P = 128
B = 512
BG = 2                  # batch groups
SH = 4                  # neuron shards
BC = B // BG            # 256 batch per core
IN = 193
NGROUP = 3
TAU = 100.0
N_CORES = 8

# layers 1/2: 16000 real neurons -> 4096 slots/shard (96 pads each)
NJ12 = 32               # j-columns per shard
REAL12 = 4000           # real neurons per shard
NS12 = NJ12 * P         # 4096 slots per shard
# layer 3: 15999 real -> 33 j-cols/shard; group g owns local j in [11g, 11g+11)
NJ3 = 33
JPG = 11                # j-cols per group per shard
NS3 = NJ3 * P           # 4224 slots per shard
SPG = 15999 // NGROUP   # 5333 real slots per group

_CACHE = {}


def _build_nc():
    import concourse.bacc as bacc
    import concourse.tile as tile
    import concourse.mybir as mybir

    f32 = mybir.dt.float32
    bf16 = mybir.dt.bfloat16
    fp8 = mybir.dt.float8e4
    i16 = mybir.dt.int16
    Alu = mybir.AluOpType
    Act = mybir.ActivationFunctionType
    Ax = mybir.AxisListType

    nc = bacc.Bacc(
        "TRN2", target_bir_lowering=False, debug=False, num_devices=N_CORES,
        num_swdge_queues=4,
    )

    # ---- I/O ----
    xT = nc.dram_tensor("xT", [IN, BC], bf16, kind="ExternalInput")
    wps = [
        nc.dram_tensor("w1p", [P, NJ12 * 16], f32, kind="ExternalInput"),
        nc.dram_tensor("w2p", [P, NJ12 * 16], f32, kind="ExternalInput"),
        nc.dram_tensor("w3p", [P, NJ3 * 16], f32, kind="ExternalInput"),
    ]
    idxs = []
    for l, ns in ((1, NS12), (2, NS12), (3, NS3)):
        # combined a+b index stream, chunk-interleaved: [a-chunk0 b-chunk0 ...]
        idxs.append(
            nc.dram_tensor(f"i{l}", [P, 2 * ns // 16], i16, kind="ExternalInput")
        )
    out_d = nc.dram_tensor("out", [1, NGROUP * BC], f32, kind="ExternalOutput")

    # collective buffers (h exchange, NCH j-chunks pipelined) and partial-sum
    # exchange.  g layout is chunk-major: row r = k*SH*P + s*P + p, unit
    # r*JCH + (j % JCH)  with JCH = NJ12//NCH j-cols per chunk.
    NCH_ = NCH
    JCH_ = JCH
    cins = [
        [
            nc.dram_tensor(f"cin{l}_{k}", [P, JCH * BC], fp8, kind="Internal")
            for k in range(NCH)
        ]
        for l in (1, 2)
    ]
    gs_ = [
        nc.dram_tensor("g1", [NCH * SH * P, JCH * BC], fp8, kind="Internal"),
        nc.dram_tensor("g2", [NCH * SH * P, JCH * BC], fp8, kind="Internal"),
    ]
    # warm-up collective: absorbs first-collective firmware latency while the
    # layer-1 gathers run.
    win = nc.dram_tensor("win", [P, 16], f32, kind="Internal")
    warm = nc.dram_tensor("warm", [SH * P, 16], f32, kind="Internal")
    pin = nc.dram_tensor("pin", [1, NGROUP * BC], f32, kind="Internal")
    pall = nc.dram_tensor("pall", [SH, NGROUP * BC], f32, kind="Internal")

    shard_groups = [[0, 1, 2, 3], [4, 5, 6, 7]]

    with tile.TileContext(nc) as tc:
        with (
            tc.tile_pool(name="big", bufs=1) as big,
            tc.tile_pool(name="prep", bufs=2) as prep,
            tc.tile_pool(name="small", bufs=2) as small,
            tc.tile_pool(name="psum", bufs=1, space="PSUM") as psum,
        ):
            layers = [
                (NJ12, NS12, bf16, bf16, fp8, xT[:], idxs[0], wps[0], cins[0], gs_[0]),
                (
                    NJ12, NS12, fp8, bf16, fp8,
                    gs_[0][:].rearrange("r (j b) -> (r j) b", b=BC),
                    idxs[1], wps[1], cins[1], gs_[1],
                ),
                (
                    NJ3, NS3, fp8, bf16, bf16,
                    gs_[1][:].rearrange("r (j b) -> (r j) b", b=BC),
                    idxs[2], wps[2], None, None,
                ),
            ]  # per layer: (NJ, NS, gather-dtype, tmp-dtype, h-dtype, src, ...)

            wsb = prep.tile([P, 16], f32, tag="wsb")
            nc.vector.memset(wsb[:], 0.0)
            nc.sync.dma_start(win[:], wsb[:])
            nc.gpsimd.collective_compute(
                "AllGather", Alu.bypass, replica_groups=shard_groups,
                ins=[win[:]], outs=[warm[:]],
            )

            h_final = None
            for li, (NJ, NS, gdt, tdt, hdt, src, iad, wp, cin, gout) in enumerate(
                layers
            ):
                last = li == 2
                # ---- coefficient prep: C0..C3 [P, NJ] f32 ----
                wt = prep.tile([P, NJ * 16], f32, tag="wt")
                nc.sync.dma_start(wt[:], wp[:])
                e = prep.tile([P, NJ * 16], f32, tag="e")
                nc.scalar.activation(e[:], wt[:], Act.Exp)
                e3 = e[:].rearrange("p (j g) -> p j g", g=16)
                e4 = e[:].rearrange("p (j h q) -> p j h q", h=4, q=4)

                ssum = small.tile([P, NJ], f32, tag="ssum")
                nc.vector.reduce_sum(ssum[:], e3, axis=Ax.X)
                r = small.tile([P, NJ], f32, tag="r")
                nc.vector.reciprocal(r[:], ssum[:])

                c0 = small.tile([P, NJ], f32, tag="c0")
                c1 = small.tile([P, NJ], f32, tag="c1")
                c2 = small.tile([P, NJ], f32, tag="c2")
                c3 = small.tile([P, NJ], f32, tag="c3")

                nc.vector.reduce_sum(c0[:], e4[:, :, 2:4, :], axis=Ax.XY)
                t1 = small.tile([P, NJ], f32, tag="t1")
                t2 = small.tile([P, NJ], f32, tag="t2")
                nc.vector.reduce_sum(t1[:], e4[:, :, 0:2, 2:4], axis=Ax.XY)
                nc.vector.reduce_sum(t2[:], e4[:, :, 2:4, 0:2], axis=Ax.XY)
                nc.vector.tensor_sub(c1[:], t1[:], t2[:])
                t3 = small.tile([P, NJ], f32, tag="t3")
                t4 = small.tile([P, NJ], f32, tag="t4")
                nc.vector.reduce_sum(t3[:], e4[:, :, 1, :], axis=Ax.X)
                nc.vector.reduce_sum(t4[:], e4[:, :, 2, :], axis=Ax.X)
                nc.vector.tensor_sub(c2[:], t3[:], t4[:])
                f = small.tile([P, NJ, 7], f32, tag="f")
                nc.vector.tensor_sub(f[:], e3[:, :, 1:8], e3[:, :, 14:7:-1])
                u1 = small.tile([P, NJ], f32, tag="u1")
                u2 = small.tile([P, NJ], f32, tag="u2")
                nc.vector.tensor_sub(u1[:], f[:, :, 0], f[:, :, 1])
                nc.vector.tensor_add(u2[:], f[:, :, 3], f[:, :, 6])
                nc.vector.tensor_sub(u1[:], u1[:], u2[:])
                nc.vector.scalar_tensor_tensor(
                    c3[:], f[:, :, 5], -2.0, u1[:], op0=Alu.mult, op1=Alu.add
                )
                for ck in (c0, c1, c2, c3):
                    nc.vector.tensor_mul(ck[:], ck[:], r[:])

                # ---- idx load ----
                iab = prep.tile([P, 2 * NS // 16], i16, tag="iab")
                nc.sync.dma_start(iab[:], iad[:])

                # ---- chunked gathers + combine ----
                h = big.tile([P, NJ * BC], hdt, tag="h")
                h3 = h[:].rearrange("p (j b) -> p j b", b=BC)
                if last:
                    # group-aligned chunks so GroupSum reduces fire per chunk
                    chunks = [(0, 11), (11, 22), (22, NJ)]
                else:
                    chunks = [(k * JCH, (k + 1) * JCH) for k in range(NCH)]
                for ci, (j0, j1) in enumerate(chunks):
                    cw = j1 - j0
                    ab = big.tile([P, 2 * cw, BC], gdt, tag=f"ab{ci}")
                    nsc = 2 * cw * P
                    nc.gpsimd.dma_gather(
                        ab[:], src, iab[:, 2 * j0 * 8 : 2 * j1 * 8], nsc, nsc, BC,
                        single_packet=False, queue_num=ci % 4,
                    )
                    tmp = big.tile([P, cw, BC], tdt, tag=f"t{ci}")
                    for j in range(j0, j1):
                        jl = j - j0
                        aj = ab[:, jl]
                        bj = ab[:, cw + jl]
                        # tmp = (a*C3)*b ; tmp = (a*C1)+tmp ; tmp = (b*C2)+tmp
                        nc.vector.scalar_tensor_tensor(
                            tmp[:, jl], aj, c3[:, j : j + 1], bj,
                            op0=Alu.mult, op1=Alu.mult,
                        )
                        nc.vector.scalar_tensor_tensor(
                            tmp[:, jl], aj, c1[:, j : j + 1], tmp[:, jl],
                            op0=Alu.mult, op1=Alu.add,
                        )
                        nc.vector.scalar_tensor_tensor(
                            tmp[:, jl], bj, c2[:, j : j + 1], tmp[:, jl],
                            op0=Alu.mult, op1=Alu.add,
                        )
                        # h = tmp + C0 on the Scalar engine
                        nc.scalar.activation(
                            h3[:, j], tmp[:, jl], Act.Identity,
                            bias=c0[:, j : j + 1], scale=1.0,
                        )

                    if not last:
                        # ship this chunk as soon as it's combined
                        nc.sync.dma_start(
                            cin[ci][:], h[:, j0 * BC : j1 * BC]
                        )
                        nc.gpsimd.collective_compute(
                            "AllGather", Alu.bypass, replica_groups=shard_groups,
                            ins=[cin[ci][:]],
                            outs=[gout[ci * SH * P : (ci + 1) * SH * P, :]],
                        )
                if last:
                    h_final = h

            # ---- GroupSum: per-shard partials, then cross-shard AllGather+sum ----
            gs = prep.tile([P, NGROUP * BC], f32, tag="gs")
            for g in range(NGROUP):
                sl = h_final[:, g * JPG * BC : (g + 1) * JPG * BC].rearrange(
                    "p (j b) -> p b j", b=BC
                )
                nc.vector.reduce_sum(gs[:, g * BC : (g + 1) * BC], sl, axis=Ax.X)
            ones = prep.tile([P, 1], f32, tag="ones")
            nc.vector.memset(ones[:], 1.0)
            psc = prep.tile([1, NGROUP * BC], f32, tag="psc")
            HW = NGROUP * BC // 2
            for k in range(2):
                ps = psum.tile([1, HW], f32, tag=f"ps{k}")
                nc.tensor.matmul(
                    ps[:], ones[:], gs[:, k * HW : (k + 1) * HW],
                    start=True, stop=True,
                )
                nc.scalar.copy(psc[:, k * HW : (k + 1) * HW], ps[:])
            nc.sync.dma_start(pin[:], psc[:])
            nc.gpsimd.collective_compute(
                "AllGather", Alu.bypass, replica_groups=shard_groups,
                ins=[pin[:]], outs=[pall[:]],
            )
            pall_sb = prep.tile([SH, NGROUP * BC], f32, tag="pall_sb")
            nc.sync.dma_start(pall_sb[:], pall[:])
            ones4 = prep.tile([SH, 1], f32, tag="ones4")
            nc.vector.memset(ones4[:], 1.0)
            osb = prep.tile([1, NGROUP * BC], f32, tag="osb")
            for k in range(2):
                ps2 = psum.tile([1, HW], f32, tag=f"ps2{k}")
                nc.tensor.matmul(
                    ps2[:], ones4[:], pall_sb[:, k * HW : (k + 1) * HW],
                    start=True, stop=True,
                )
                nc.scalar.mul(osb[:, k * HW : (k + 1) * HW], ps2[:], 1.0 / TAU)
            # consume the warm-up collective's (all-zero) output so DCE keeps it
            wsb2 = prep.tile([1, 16], f32, tag="wsb2")
            nc.sync.dma_start(wsb2[:], warm[0:1, :])
            nc.vector.tensor_add(osb[:, :16], osb[:, :16], wsb2[:])
            nc.sync.dma_start(out_d[:], osb[:])

    nc.compile()
    return nc


def _wrap_idx(ii):
    w = ii.astype(np.int16).reshape(-1, 16).T
    return np.ascontiguousarray(np.tile(w, (8, 1)))


CHUNKS12 = [(0, 8), (8, 16), (16, 24), (24, 32)]
CHUNKS3 = [(0, 11), (11, 22), (22, 33)]


def _combine_idx(ia_eff, ib_eff, chunk_list):
    """Interleave a/b index streams per chunk: [a-chunk0, b-chunk0, a-chunk1, ...]"""
    parts = []
    for j0, j1 in chunk_list:
        parts.append(ia_eff[j0 * P : j1 * P])
        parts.append(ib_eff[j0 * P : j1 * P])
    return _wrap_idx(np.concatenate(parts))


def _pack_w(w_eff, nj):
    # local slot t = j*128 + p  ->  packed[p, j*16+g]
    return np.ascontiguousarray(
        w_eff.reshape(nj, P, 16).transpose(1, 0, 2).reshape(P, nj * 16)
    )


NCH = 4
JCH = NJ12 // NCH


def _src_unit12(i):
    """BC-row unit of layer-1/2 neuron i in the chunk-major AllGathered
    [NCH*SH*128, JCH*BC] layout: shard s = i//4000, local t = i - 4000s,
    p = t%128, j = t//128, chunk k = j//JCH; row = (k*SH+s)*128+p,
    unit = row*JCH + j%JCH."""
    s = i // REAL12
    t = i - s * REAL12
    p = t % P
    j = t // P
    k = j // JCH
    return ((k * SH + s) * P + p) * JCH + j % JCH


def _host_pack(inputs):
    x = np.asarray(inputs["x"], dtype=np.float32)
    w1 = np.asarray(inputs["w1"], dtype=np.float32)
    w2 = np.asarray(inputs["w2"], dtype=np.float32)
    w3 = np.asarray(inputs["w3"], dtype=np.float32)
    i1a = np.asarray(inputs["idx1a"]).astype(np.int64)
    i1b = np.asarray(inputs["idx1b"]).astype(np.int64)
    i2a = np.asarray(inputs["idx2a"]).astype(np.int64)
    i2b = np.asarray(inputs["idx2b"]).astype(np.int64)
    i3a = np.asarray(inputs["idx3a"]).astype(np.int64)
    i3b = np.asarray(inputs["idx3b"]).astype(np.int64)

    pad_row = np.full(16, -20.0, dtype=np.float32)
    pad_row[0] = 20.0  # softmax -> ~one-hot FALSE gate -> h = 0

    per_shard = [dict() for _ in range(SH)]
    # layers 1 and 2: shard s owns real neurons [s*4000, (s+1)*4000)
    for l, (w, ja, jb, srcf) in enumerate(
        (
            (w1, i1a, i1b, lambda i: i),
            (w2, i2a, i2b, _src_unit12),
        ),
        start=1,
    ):
        for s in range(SH):
            sel = slice(s * REAL12, (s + 1) * REAL12)
            w_eff = np.concatenate(
                [w[sel], np.tile(pad_row, (NS12 - REAL12, 1))], axis=0
            )
            ia_eff = np.zeros(NS12, dtype=np.int64)
            ib_eff = np.zeros(NS12, dtype=np.int64)
            ia_eff[:REAL12] = srcf(ja[sel])
            ib_eff[:REAL12] = srcf(jb[sel])
            per_shard[s][f"w{l}p"] = _pack_w(w_eff, NJ12)
            per_shard[s][f"i{l}"] = _combine_idx(ia_eff, ib_eff, CHUNKS12)

    # layer 3: group g's 5333 real neurons split over shards as
    # counts c_s = [1334, 1333, 1333, 1333]; within (s, g): local j in
    # [11g, 11g+11), rank m = (j-11g)*128 + p
    counts = np.array([1334, 1333, 1333, 1333])
    offs = np.concatenate([[0], np.cumsum(counts)[:-1]])
    u = np.arange(NS3)
    jj = u // P
    pp = u % P
    gg = jj // JPG
    m = (jj - gg * JPG) * P + pp
    for s in range(SH):
        real = m < counts[s]
        rid = gg * SPG + offs[s] + np.minimum(m, counts[s] - 1)
        w3_eff = w3[rid].copy()
        w3_eff[~real] = pad_row
        i3a_eff = np.where(real, _src_unit12(i3a[rid]), 0)
        i3b_eff = np.where(real, _src_unit12(i3b[rid]), 0)
        per_shard[s]["w3p"] = _pack_w(w3_eff, NJ3)
        per_shard[s]["i3"] = _combine_idx(i3a_eff, i3b_eff, CHUNKS3)

    import ml_dtypes

    in_maps = []
    for c in range(N_CORES):
        G, s = c // SH, c % SH
        m_ = dict(per_shard[s])
        m_["xT"] = np.ascontiguousarray(
            x[G * BC : (G + 1) * BC].T.astype(ml_dtypes.bfloat16)
        )
        in_maps.append(m_)
    return in_maps


LAST_RESULTS = None


def kernel(**inputs):
    global LAST_RESULTS
    from concourse.bass_utils import run_bass_kernel_spmd

    if "nc" not in _CACHE:
        _CACHE["nc"] = _build_nc()
    nc = _CACHE["nc"]

    in_maps = _host_pack(inputs)
    trace = bool(int(os.environ.get("KERNEL_TRACE", "0")))
    res = run_bass_kernel_spmd(
        nc, in_maps, core_ids=list(range(N_CORES)), trace=trace
    )
    LAST_RESULTS = res

    out = np.empty((B, NGROUP), dtype=np.float32)
    for g_ in range(BG):
        rc = res.results[g_ * SH]["out"].reshape(NGROUP, BC)
        out[g_ * BC : (g_ + 1) * BC, :] = rc.T
    return out



# revision 26
# speedup vs baseline: 1.1070x; 1.0581x over previous
"""Trainium2 Bass kernel for a 3-layer difflogic network (nn_Net_48610439856713).

Math: each layer o computes softmax(w[o])·ops16(a, b) with a = h[:, ia[o]],
b = h[:, ib[o]].  The 16 relaxed logic gates are all affine in {1, a, b, ab},
so the layer reduces to  h' = C0 + C1·a + C2·b + C3·a·b  with 4 per-neuron
coefficients derived on-device from softmax(w).

Sharding: 2 batch groups x 4 neuron shards over the 8 cores.  Core c handles
batch rows [(c//4)*256, ...) and neuron shard c%4 of every layer.  Activations
are fp8e4 in a transposed packed layout h^T[neuron, batch]; each layer's shard
outputs are exchanged with a 4-rank AllGather, chunked so later layers can
start early: layer l+1's neurons are bucket-sorted by the AllGather chunk in
which their last-arriving source lands, so bucket k's gathers fire as soon as
chunk k of the previous layer's exchange completes (expressed to the Tile
scheduler via row-restricted gather source APs).  Gathers run on 4 parallel
SWDGE queues (the per-queue descriptor processing rate, ~8ns/row, is the
limiter, so queue parallelism quarters gather time).

Host-side bookkeeping is integer/layout only: slot permutations, bucket
sorting, index relabeling, int16 index wrapping, weight-row packing.  All
float arithmetic (softmax, combine, sums) runs on device.
"""

import os
import numpy as np

P = 128
B = 512
BG = 2                  # batch groups
SH = 4                  # neuron shards
BC = B // BG            # 256 batch per core
IN = 193
NGROUP = 3
TAU = 100.0
N_CORES = 8

NJ1 = 32                # layer-1 j-columns per shard (4096 slots, 96 pads)
REAL12 = 4000           # real neurons per shard, layers 1-2
NCH = 4                 # AllGather chunks per exchange == dependency buckets
W1 = NJ1 // NCH         # layer-1 AllGather chunk width (cols)
SPG = 15999 // NGROUP   # 5333 layer-3 neurons per output group

_CACHE = {}


def _build_nc(shapes):
    """shapes: dict with w2k (4 L2 bucket widths), w2ch (AG2 chunk width),
    nj2 (total L2 cols), wkg (4x3 L3 segment widths), nj3 (total L3 cols)."""
    import concourse.bacc as bacc
    import concourse.tile as tile
    import concourse.mybir as mybir

    f32 = mybir.dt.float32
    bf16 = mybir.dt.bfloat16
    fp8 = mybir.dt.float8e4
    i16 = mybir.dt.int16
    Alu = mybir.AluOpType
    Act = mybir.ActivationFunctionType
    Ax = mybir.AxisListType

    w2k = shapes["w2k"]
    W2 = shapes["w2ch"]
    NJ2 = shapes["nj2"]
    wkg = shapes["wkg"]
    NJ3 = shapes["nj3"]

    nc = bacc.Bacc(
        "TRN2", target_bir_lowering=False, debug=False, num_devices=N_CORES,
        num_swdge_queues=4,
    )

    # ---- I/O ----
    xT = nc.dram_tensor("xT", [IN, BC], bf16, kind="ExternalInput")
    wps = [
        nc.dram_tensor("w1p", [P, NJ1 * 16], f32, kind="ExternalInput"),
        nc.dram_tensor("w2p", [P, NJ2 * 16], f32, kind="ExternalInput"),
        nc.dram_tensor("w3p", [P, NJ3 * 16], f32, kind="ExternalInput"),
    ]
    idxs = [
        nc.dram_tensor(f"i{l}", [P, 2 * nj * P // 16], i16, kind="ExternalInput")
        for l, nj in ((1, NJ1), (2, NJ2), (3, NJ3))
    ]
    out_d = nc.dram_tensor("out", [1, NGROUP * BC], f32, kind="ExternalOutput")

    # collective buffers.  g layout is chunk-major: row r = k*SH*P + s*P + p,
    # unit r*W + (j - k*W) for the layer's chunk width W.
    cins = [
        [
            nc.dram_tensor(f"cin1_{k}", [P, W1 * BC], fp8, kind="Internal")
            for k in range(NCH)
        ],
        [
            nc.dram_tensor(f"cin2_{k}", [P, W2 * BC], fp8, kind="Internal")
            for k in range(NCH)
        ],
    ]
    gs_ = [
        nc.dram_tensor("g1", [NCH * SH * P, W1 * BC], fp8, kind="Internal"),
        nc.dram_tensor("g2", [NCH * SH * P, W2 * BC], fp8, kind="Internal"),
    ]
    # warm-up collective: absorbs first-collective firmware latency.
    win = nc.dram_tensor("win", [P, 16], f32, kind="Internal")
    warm = nc.dram_tensor("warm", [SH * P, 16], f32, kind="Internal")
    pin = nc.dram_tensor("pin", [1, NGROUP * BC], f32, kind="Internal")
    pall = nc.dram_tensor("pall", [SH, NGROUP * BC], f32, kind="Internal")

    shard_groups = [[0, 1, 2, 3], [4, 5, 6, 7]]

    def bucket_cols(widths):
        """[(J0, J1, bucket_k)] from per-bucket widths."""
        out, j = [], 0
        for k, w in enumerate(widths):
            out.append((j, j + w, k))
            j += w
        return out

    with tile.TileContext(nc) as tc:
        with (
            tc.tile_pool(name="big", bufs=1) as big,
            tc.tile_pool(name="prep", bufs=2) as prep,
            tc.tile_pool(name="small", bufs=2) as small,
            tc.tile_pool(name="psum", bufs=1, space="PSUM") as psum,
        ):
            wsb = prep.tile([P, 16], f32, tag="wsb")
            nc.vector.memset(wsb[:], 0.0)
            nc.sync.dma_start(win[:], wsb[:])
            nc.gpsimd.collective_compute(
                "AllGather", Alu.bypass, replica_groups=shard_groups,
                ins=[win[:]], outs=[warm[:]],
            )

            # layer-3 per-(group,bucket) partial sums [P, g*4+k, BC]
            pk = prep.tile([P, NGROUP * NCH, BC], f32, tag="pk")
            nc.vector.memset(pk[:], 0.0)

            # per layer: (NJ, gather dt, tmp dt, h dt, src getter, idx, w,
            #             cin list, gout, AG chunk width, items)
            # src getter(k) -> AP restricted to rows available after dep
            # chunk k has landed (None = no restriction).
            def src1(_k):
                return xT[:]

            def src2(k):
                return gs_[0][0 : (k + 1) * SH * P, :].rearrange(
                    "r (j b) -> (r j) b", b=BC
                )

            def src3(k):
                return gs_[1][0 : (k + 1) * SH * P, :].rearrange(
                    "r (j b) -> (r j) b", b=BC
                )

            layers = [
                (NJ1, bf16, bf16, fp8, src1, idxs[0], wps[0], cins[0], gs_[0],
                 W1, bucket_cols([W1] * NCH), None),
                (NJ2, fp8, bf16, fp8, src2, idxs[1], wps[1], cins[1], gs_[1],
                 W2, bucket_cols(w2k), None),
                (NJ3, fp8, bf16, bf16, src3, idxs[2], wps[2], None, None,
                 None, bucket_cols([sum(w) for w in wkg]), wkg),
            ]

            qn = [0]  # round-robin SWDGE queue counter

            for li, (NJ, gdt, tdt, hdt, srcf, iad, wp, cin, gout, WCH, items,
                     segs) in enumerate(layers):
                last = li == 2
                # ---- coefficient prep: C0..C3 [P, NJ] f32 ----
                wt = prep.tile([P, NJ * 16], f32, tag="wt")
                nc.sync.dma_start(wt[:], wp[:])
                e = prep.tile([P, NJ * 16], f32, tag="e")
                nc.scalar.activation(e[:], wt[:], Act.Exp)
                e3 = e[:].rearrange("p (j g) -> p j g", g=16)
                e4 = e[:].rearrange("p (j h q) -> p j h q", h=4, q=4)

                ssum = small.tile([P, NJ], f32, tag="ssum")
                nc.vector.reduce_sum(ssum[:], e3, axis=Ax.X)
                r = small.tile([P, NJ], f32, tag="r")
                nc.vector.reciprocal(r[:], ssum[:])

                c0 = small.tile([P, NJ], f32, tag="c0")
                c1 = small.tile([P, NJ], f32, tag="c1")
                c2 = small.tile([P, NJ], f32, tag="c2")
                c3 = small.tile([P, NJ], f32, tag="c3")

                nc.vector.reduce_sum(c0[:], e4[:, :, 2:4, :], axis=Ax.XY)
                t1 = small.tile([P, NJ], f32, tag="t1")
                t2 = small.tile([P, NJ], f32, tag="t2")
                nc.vector.reduce_sum(t1[:], e4[:, :, 0:2, 2:4], axis=Ax.XY)
                nc.vector.reduce_sum(t2[:], e4[:, :, 2:4, 0:2], axis=Ax.XY)
                nc.vector.tensor_sub(c1[:], t1[:], t2[:])
                t3 = small.tile([P, NJ], f32, tag="t3")
                t4 = small.tile([P, NJ], f32, tag="t4")
                nc.vector.reduce_sum(t3[:], e4[:, :, 1, :], axis=Ax.X)
                nc.vector.reduce_sum(t4[:], e4[:, :, 2, :], axis=Ax.X)
                nc.vector.tensor_sub(c2[:], t3[:], t4[:])
                f = small.tile([P, NJ, 7], f32, tag="f")
                nc.vector.tensor_sub(f[:], e3[:, :, 1:8], e3[:, :, 14:7:-1])
                u1 = small.tile([P, NJ], f32, tag="u1")
                u2 = small.tile([P, NJ], f32, tag="u2")
                nc.vector.tensor_sub(u1[:], f[:, :, 0], f[:, :, 1])
                nc.vector.tensor_add(u2[:], f[:, :, 3], f[:, :, 6])
                nc.vector.tensor_sub(u1[:], u1[:], u2[:])
                nc.vector.scalar_tensor_tensor(
                    c3[:], f[:, :, 5], -2.0, u1[:], op0=Alu.mult, op1=Alu.add
                )
                for ck in (c0, c1, c2, c3):
                    nc.vector.tensor_mul(ck[:], ck[:], r[:])

                # ---- idx load ----
                iab = prep.tile([P, 2 * NJ * 8], i16, tag="iab")
                nc.sync.dma_start(iab[:], iad[:])

                # ---- bucketed gathers + combine ----
                if not last:
                    h = big.tile([P, NJ * BC], hdt, tag="h")
                    h3 = h[:].rearrange("p (j b) -> p j b", b=BC)
                shipped = 0  # AG chunks shipped so far

                for bi, (j0, j1, dep) in enumerate(items):
                    cw = j1 - j0
                    if cw == 0:
                        continue
                    src = srcf(dep)
                    ab = big.tile([P, 2 * cw, BC], gdt, tag=f"ab{bi}")
                    if last:
                        h3b = big.tile([P, cw, BC], hdt, tag=f"h3{bi % 2}")
                    # a-rows then b-rows; each stream split in two gather
                    # calls when wide, across the 4 SWDGE queues.
                    for half, coff in ((0, 0), (1, cw)):
                        nsplit = 2 if cw >= 8 else 1
                        csp = [
                            (cw * t // nsplit, cw * (t + 1) // nsplit)
                            for t in range(nsplit)
                        ]
                        for (s0, s1) in csp:
                            nr = (s1 - s0) * P
                            if nr == 0:
                                continue
                            ioff = 2 * j0 * 8 + half * cw * 8 + s0 * 8
                            nc.gpsimd.dma_gather(
                                ab[:, coff + s0 : coff + s1], src,
                                iab[:, ioff : ioff + (s1 - s0) * 8],
                                nr, nr, BC,
                                single_packet=False, queue_num=qn[0] % 4,
                            )
                            qn[0] += 1
                    tmp = big.tile([P, cw, BC], tdt, tag=f"t{bi}")
                    for j in range(j0, j1):
                        jl = j - j0
                        aj = ab[:, jl]
                        bj = ab[:, cw + jl]
                        # tmp = (a*C3)*b ; tmp = (a*C1)+tmp ; tmp = (b*C2)+tmp
                        nc.vector.scalar_tensor_tensor(
                            tmp[:, jl], aj, c3[:, j : j + 1], bj,
                            op0=Alu.mult, op1=Alu.mult,
                        )
                        nc.vector.scalar_tensor_tensor(
                            tmp[:, jl], aj, c1[:, j : j + 1], tmp[:, jl],
                            op0=Alu.mult, op1=Alu.add,
                        )
                        nc.vector.scalar_tensor_tensor(
                            tmp[:, jl], bj, c2[:, j : j + 1], tmp[:, jl],
                            op0=Alu.mult, op1=Alu.add,
                        )
                        # h = tmp + C0 on the Scalar engine
                        hout = h3b[:, jl] if last else h3[:, j]
                        nc.scalar.activation(
                            hout, tmp[:, jl], Act.Identity,
                            bias=c0[:, j : j + 1], scale=1.0,
                        )

                    if last:
                        # GroupSum partials for this bucket's segments
                        k = items[bi][2] if False else bi  # bucket index
                        c = 0
                        for g in range(NGROUP):
                            wseg = segs[bi][g]
                            if wseg == 0:
                                continue
                            sl = h3b[:, c : c + wseg].rearrange(
                                "p j b -> p b j"
                            )
                            nc.vector.reduce_sum(
                                pk[:, g * NCH + bi], sl, axis=Ax.X
                            )
                            c += wseg
                    else:
                        # ship completed AG chunks
                        while (shipped + 1) * WCH <= j1:
                            m = shipped
                            nc.sync.dma_start(
                                cin[m][:], h[:, m * WCH * BC : (m + 1) * WCH * BC]
                            )
                            nc.gpsimd.collective_compute(
                                "AllGather", Alu.bypass,
                                replica_groups=shard_groups,
                                ins=[cin[m][:]],
                                outs=[gout[m * SH * P : (m + 1) * SH * P, :]],
                            )
                            shipped += 1

            # ---- GroupSum: fold buckets, then cross-shard AllGather+sum ----
            gs = prep.tile([P, NGROUP * BC], f32, tag="gs")
            pkv = pk[:].rearrange("p (g k) b -> p g b k", g=NGROUP, k=NCH)
            gsv = gs[:].rearrange("p (g b) -> p g b", b=BC)
            nc.vector.reduce_sum(gsv, pkv, axis=Ax.X)
            ones = prep.tile([P, 1], f32, tag="ones")
            nc.vector.memset(ones[:], 1.0)
            psc = prep.tile([1, NGROUP * BC], f32, tag="psc")
            HW = NGROUP * BC // 2
            for k in range(2):
                ps = psum.tile([1, HW], f32, tag=f"ps{k}")
                nc.tensor.matmul(
                    ps[:], ones[:], gs[:, k * HW : (k + 1) * HW],
                    start=True, stop=True,
                )
                nc.scalar.copy(psc[:, k * HW : (k + 1) * HW], ps[:])
            nc.sync.dma_start(pin[:], psc[:])
            nc.gpsimd.collective_compute(
                "AllGather", Alu.bypass, replica_groups=shard_groups,
                ins=[pin[:]], outs=[pall[:]],
            )
            pall_sb = prep.tile([SH, NGROUP * BC], f32, tag="pall_sb")
            nc.sync.dma_start(pall_sb[:], pall[:])
            ones4 = prep.tile([SH, 1], f32, tag="ones4")
            nc.vector.memset(ones4[:], 1.0)
            osb = prep.tile([1, NGROUP * BC], f32, tag="osb")
            for k in range(2):
                ps2 = psum.tile([1, HW], f32, tag=f"ps2{k}")
                nc.tensor.matmul(
                    ps2[:], ones4[:], pall_sb[:, k * HW : (k + 1) * HW],
                    start=True, stop=True,
                )
                nc.scalar.mul(osb[:, k * HW : (k + 1) * HW], ps2[:], 1.0 / TAU)
            # consume the warm-up collective's (all-zero) output so DCE keeps it
            wsb2 = prep.tile([1, 16], f32, tag="wsb2")
            nc.sync.dma_start(wsb2[:], warm[0:1, :])
            nc.vector.tensor_add(osb[:, :16], osb[:, :16], wsb2[:])
            nc.sync.dma_start(out_d[:], osb[:])

    nc.compile()
    return nc


def _wrap_idx(ii):
    w = ii.astype(np.int16).reshape(-1, 16).T
    return np.ascontiguousarray(np.tile(w, (8, 1)))


def _pack_w(w_eff, nj):
    # local slot t = j*128 + p  ->  packed[p, j*16+g]
    return np.ascontiguousarray(
        w_eff.reshape(nj, P, 16).transpose(1, 0, 2).reshape(P, nj * 16)
    )


def _chunk1(i):
    """AG1 chunk of original layer-1 neuron id i."""
    s = i // REAL12
    t = i - s * REAL12
    return (t // P) // W1


def _src_unit1(i):
    """gs_[0] row-unit of layer-1 neuron i (chunk-major, chunk width W1)."""
    s = i // REAL12
    t = i - s * REAL12
    p = t % P
    j = t // P
    k = j // W1
    return ((k * SH + s) * P + p) * W1 + j % W1


PAD16 = None  # set in _host_pack


def _host_pack(inputs):
    x = np.asarray(inputs["x"], dtype=np.float32)
    w1 = np.asarray(inputs["w1"], dtype=np.float32)
    w2 = np.asarray(inputs["w2"], dtype=np.float32)
    w3 = np.asarray(inputs["w3"], dtype=np.float32)
    i1a = np.asarray(inputs["idx1a"]).astype(np.int64)
    i1b = np.asarray(inputs["idx1b"]).astype(np.int64)
    i2a = np.asarray(inputs["idx2a"]).astype(np.int64)
    i2b = np.asarray(inputs["idx2b"]).astype(np.int64)
    i3a = np.asarray(inputs["idx3a"]).astype(np.int64)
    i3b = np.asarray(inputs["idx3b"]).astype(np.int64)

    pad_row = np.full(16, -20.0, dtype=np.float32)
    pad_row[0] = 20.0  # softmax -> ~one-hot FALSE gate -> h = 0

    per_shard = [dict() for _ in range(SH)]

    # ---- layer 1: shard s owns neurons [s*4000, (s+1)*4000), slot = rank;
    # 4 fixed buckets of 8 cols; idx stream = [a-cols | b-cols] per bucket.
    for s in range(SH):
        sel = slice(s * REAL12, (s + 1) * REAL12)
        NS1 = NJ1 * P
        w_eff = np.concatenate(
            [w1[sel], np.tile(pad_row, (NS1 - REAL12, 1))], axis=0
        )
        ia_eff = np.zeros(NS1, dtype=np.int64)
        ib_eff = np.zeros(NS1, dtype=np.int64)
        ia_eff[:REAL12] = i1a[sel]
        ib_eff[:REAL12] = i1b[sel]
        parts = []
        for k in range(NCH):
            lo, hi = k * W1 * P, (k + 1) * W1 * P
            parts.append(ia_eff[lo:hi])
            parts.append(ib_eff[lo:hi])
        per_shard[s]["w1p"] = _pack_w(w_eff, NJ1)
        per_shard[s]["i1"] = _wrap_idx(np.concatenate(parts))

    # ---- layer 2: bucket-sort each shard's neurons by max source AG1 chunk.
    # Uniform (max-over-shard) bucket widths keep the single SPMD program.
    ord2, cnt2 = [], np.zeros((SH, NCH), dtype=np.int64)
    for s in range(SH):
        sel = np.arange(s * REAL12, (s + 1) * REAL12)
        m = np.maximum(_chunk1(i2a[sel]), _chunk1(i2b[sel]))
        o = np.argsort(m, kind="stable")
        ord2.append(sel[o])
        cnt2[s] = np.bincount(m, minlength=NCH)
    w2k = [int(np.ceil(cnt2[:, k].max() / P)) for k in range(NCH)]
    nj2 = sum(w2k)
    pad2 = (-nj2) % NCH
    w2k[-1] += pad2
    nj2 += pad2
    w2ch = nj2 // NCH

    # slot tables for layer-3 relabeling: neuron id -> (shard, col, p)
    slot2 = np.full(SH * REAL12, -1, dtype=np.int64)  # id -> packed slot code
    for s in range(SH):
        ids = ord2[s]
        cnts = cnt2[s]
        slot = np.zeros(len(ids), dtype=np.int64)
        pos = 0  # position within ids
        scol = 0  # starting col of bucket
        for k in range(NCH):
            n = cnts[k]
            slot[pos : pos + n] = scol * P + np.arange(n)
            pos += n
            scol += w2k[k]
        slot2[ids] = s * (nj2 * P) + slot
    # gs_[1] unit + AG2 ship chunk for each layer-2 neuron id
    s2 = slot2 // (nj2 * P)
    t2 = slot2 % (nj2 * P)
    j2 = t2 // P
    p2 = t2 % P
    m2 = j2 // w2ch
    unit2_tab = ((m2 * SH + s2) * P + p2) * w2ch + (j2 - m2 * w2ch)

    for s in range(SH):
        NS2 = nj2 * P
        ids = ord2[s]
        cnts = cnt2[s]
        w_eff = np.tile(pad_row, (NS2, 1))
        ia_eff = np.zeros(NS2, dtype=np.int64)
        ib_eff = np.zeros(NS2, dtype=np.int64)
        parts = []
        pos, scol = 0, 0
        for k in range(NCH):
            n = int(cnts[k])
            wk = w2k[k]
            lo = scol * P
            bsl = ids[pos : pos + n]
            w_eff[lo : lo + n] = w2[bsl]
            ia_eff[lo : lo + n] = _src_unit1(i2a[bsl])
            ib_eff[lo : lo + n] = _src_unit1(i2b[bsl])
            parts.append(ia_eff[lo : lo + wk * P])
            parts.append(ib_eff[lo : lo + wk * P])
            pos += n
            scol += wk
        per_shard[s]["w2p"] = _pack_w(w_eff, nj2)
        per_shard[s]["i2"] = _wrap_idx(np.concatenate(parts))

    def unit2(i):
        return unit2_tab[i]

    def chunk2(i):
        return m2[i]

    # ---- layer 3: shard split per group (counts 1334,1333,1333,1333), then
    # bucket by max AG2 ship chunk, segmented by (bucket, group).
    counts3 = np.array([1334, 1333, 1333, 1333])
    offs3 = np.concatenate([[0], np.cumsum(counts3)[:-1]])
    ids3 = []  # per shard: dict (k,g) -> neuron ids
    cnt3 = np.zeros((SH, NCH, NGROUP), dtype=np.int64)
    for s in range(SH):
        d = {}
        for g in range(NGROUP):
            rid = g * SPG + offs3[s] + np.arange(counts3[s])
            m = np.maximum(chunk2(i3a[rid]), chunk2(i3b[rid]))
            for k in range(NCH):
                d[(k, g)] = rid[m == k]
                cnt3[s, k, g] = len(d[(k, g)])
        ids3.append(d)
    wkg = [
        [int(np.ceil(cnt3[:, k, g].max() / P)) for g in range(NGROUP)]
        for k in range(NCH)
    ]
    nj3 = sum(sum(w) for w in wkg)

    for s in range(SH):
        NS3 = nj3 * P
        w_eff = np.tile(pad_row, (NS3, 1))
        ia_eff = np.zeros(NS3, dtype=np.int64)
        ib_eff = np.zeros(NS3, dtype=np.int64)
        parts = []
        scol = 0
        for k in range(NCH):
            bcol0 = scol
            for g in range(NGROUP):
                rid = ids3[s][(k, g)]
                n = len(rid)
                lo = scol * P
                w_eff[lo : lo + n] = w3[rid]
                ia_eff[lo : lo + n] = unit2(i3a[rid])
                ib_eff[lo : lo + n] = unit2(i3b[rid])
                scol += wkg[k][g]
            blo, bhi = bcol0 * P, scol * P
            parts.append(ia_eff[blo:bhi])
            parts.append(ib_eff[blo:bhi])
        per_shard[s]["w3p"] = _pack_w(w_eff, nj3)
        per_shard[s]["i3"] = _wrap_idx(np.concatenate(parts))

    shapes = dict(w2k=w2k, w2ch=w2ch, nj2=nj2, wkg=wkg, nj3=nj3)

    import ml_dtypes

    in_maps = []
    for c in range(N_CORES):
        G, s = c // SH, c % SH
        m_ = dict(per_shard[s])
        m_["xT"] = np.ascontiguousarray(
            x[G * BC : (G + 1) * BC].T.astype(ml_dtypes.bfloat16)
        )
        in_maps.append(m_)
    return in_maps, shapes


LAST_RESULTS = None


def kernel(**inputs):
    global LAST_RESULTS
    from concourse.bass_utils import run_bass_kernel_spmd

    in_maps, shapes = _host_pack(inputs)
    if "nc" not in _CACHE:
        _CACHE["nc"] = _build_nc(shapes)
    nc = _CACHE["nc"]

    trace = bool(int(os.environ.get("KERNEL_TRACE", "0")))
    res = run_bass_kernel_spmd(
        nc, in_maps, core_ids=list(range(N_CORES)), trace=trace
    )
    LAST_RESULTS = res

    out = np.empty((B, NGROUP), dtype=np.float32)
    for g_ in range(BG):
        rc = res.results[g_ * SH]["out"].reshape(NGROUP, BC)
        out[g_ * BC : (g_ + 1) * BC, :] = rc.T
    return out


# revision 28
# speedup vs baseline: 1.3371x; 1.2079x over previous
"""Trainium2 Bass kernel for a 3-layer difflogic network (nn_Net_48610439856713).

Math: each layer o computes softmax(w[o])·ops16(a, b) with a = h[:, ia[o]],
b = h[:, ib[o]].  The 16 relaxed logic gates are all affine in {1, a, b, ab},
so the layer reduces to  h' = C0 + C1·a + C2·b + C3·a·b  with 4 per-neuron
coefficients derived on-device from softmax(w).

Sharding: 2 batch groups x 4 neuron shards over the 8 cores.  Core c handles
batch rows [(c//4)*256, ...) and neuron shard c%4 of every layer.  Activations
are fp8e4 in a transposed packed layout h^T[neuron, batch]; each layer's shard
outputs are exchanged with a 4-rank AllGather, chunked with sqrt-spaced widths
(16/7/5/4 of 32) so that the next layer's dependency buckets come out uniform:
layer l+1's neurons are bucket-sorted by the AllGather chunk in which their
last-arriving source lands, and bucket k's gathers fire as soon as chunk k of
the exchange completes (expressed to the Tile scheduler via range-restricted
gather source APs).  Gathers run on 4 parallel SWDGE queues (the per-queue
descriptor processing rate, ~8ns/row, is the limiter).

Host-side bookkeeping is integer/layout only: slot permutations, bucket
sorting, index relabeling, int16 index wrapping, weight-row packing.  All
float arithmetic (softmax, combine, sums) runs on device.
"""

import os
import numpy as np

P = 128
B = 512
BG = 2                  # batch groups
SH = 4                  # neuron shards
BC = B // BG            # 256 batch per core
IN = 193
NGROUP = 3
TAU = 100.0
N_CORES = 8

NJ1 = 32                # layer-1 j-columns per shard (4096 slots, 96 pads)
REAL12 = 4000           # real neurons per shard, layers 1-2
NCH = 4                 # AllGather chunks per exchange == dependency buckets
W1SPLIT = [16, 7, 5, 4]  # sqrt-spaced AG1 chunk widths (cols)
SPG = 15999 // NGROUP   # 5333 layer-3 neurons per output group

_CACHE = {}


def _sqrt_split(nj):
    """NCH chunk widths ~ sqrt-spaced cumulative boundaries over nj cols."""
    bounds = [int(round(np.sqrt((k + 1) / NCH) * nj)) for k in range(NCH)]
    bounds[-1] = nj
    for k in range(1, NCH):
        bounds[k] = max(bounds[k], bounds[k - 1])
    widths, prev = [], 0
    for b_ in bounds:
        widths.append(b_ - prev)
        prev = b_
    return widths


def _build_nc(shapes):
    import concourse.bacc as bacc
    import concourse.tile as tile
    import concourse.mybir as mybir

    f32 = mybir.dt.float32
    bf16 = mybir.dt.bfloat16
    fp8 = mybir.dt.float8e4
    i16 = mybir.dt.int16
    Alu = mybir.AluOpType
    Act = mybir.ActivationFunctionType
    Ax = mybir.AxisListType

    w2k = shapes["w2k"]          # L2 bucket widths (4)
    NJ2 = shapes["nj2"]
    W2SPLIT = shapes["w2split"]  # AG2 chunk widths (4)
    wkg = shapes["wkg"]          # L3 per-(bucket, group) widths
    NJ3 = shapes["nj3"]

    nc = bacc.Bacc(
        "TRN2", target_bir_lowering=False, debug=False, num_devices=N_CORES,
        num_swdge_queues=4,
    )

    # ---- I/O ----
    xT = nc.dram_tensor("xT", [IN, BC], bf16, kind="ExternalInput")
    wps = [
        nc.dram_tensor("w1p", [P, NJ1 * 16], f32, kind="ExternalInput"),
        nc.dram_tensor("w2p", [P, NJ2 * 16], f32, kind="ExternalInput"),
        nc.dram_tensor("w3p", [P, NJ3 * 16], f32, kind="ExternalInput"),
    ]
    idxs = [
        nc.dram_tensor(f"i{l}", [P, 2 * nj * P // 16], i16, kind="ExternalInput")
        for l, nj in ((1, NJ1), (2, NJ2), (3, NJ3))
    ]
    out_d = nc.dram_tensor("out", [1, NGROUP * BC], f32, kind="ExternalOutput")

    # collective buffers, flat layout: chunk k occupies units
    # [SH*P*cumW[k], SH*P*cumW[k+1]) of BC elems; within chunk: shard-major,
    # then p-major, then j.
    cins = [
        [
            nc.dram_tensor(f"cin1_{k}", [P, w * BC], fp8, kind="Internal")
            for k, w in enumerate(W1SPLIT)
        ],
        [
            nc.dram_tensor(f"cin2_{k}", [P, w * BC], fp8, kind="Internal")
            for k, w in enumerate(W2SPLIT)
        ],
    ]
    gs_ = [
        nc.dram_tensor("g1", [1, SH * P * NJ1 * BC], fp8, kind="Internal"),
        nc.dram_tensor("g2", [1, SH * P * NJ2 * BC], fp8, kind="Internal"),
    ]
    # warm-up collective: absorbs first-collective firmware latency.
    win = nc.dram_tensor("win", [P, 16], f32, kind="Internal")
    warm = nc.dram_tensor("warm", [SH * P, 16], f32, kind="Internal")
    pin = nc.dram_tensor("pin", [1, NGROUP * BC], f32, kind="Internal")
    pall = nc.dram_tensor("pall", [SH, NGROUP * BC], f32, kind="Internal")

    shard_groups = [[0, 1, 2, 3], [4, 5, 6, 7]]

    def cums(ws):
        out = [0]
        for w in ws:
            out.append(out[-1] + w)
        return out

    cumW1 = cums(W1SPLIT)
    cumW2 = cums(W2SPLIT)

    def bucket_cols(widths, deps=None):
        out, j = [], 0
        for k, w in enumerate(widths):
            out.append((j, j + w, None if deps is None else deps[k]))
            j += w
        return out

    with tile.TileContext(nc) as tc:
        with (
            tc.tile_pool(name="big", bufs=1) as big,
            tc.tile_pool(name="prep", bufs=2) as prep,
            tc.tile_pool(name="small", bufs=2) as small,
            tc.tile_pool(name="psum", bufs=1, space="PSUM") as psum,
        ):
            wsb = prep.tile([P, 16], f32, tag="wsb")
            nc.vector.memset(wsb[:], 0.0)
            nc.sync.dma_start(win[:], wsb[:])
            nc.gpsimd.collective_compute(
                "AllGather", Alu.bypass, replica_groups=shard_groups,
                ins=[win[:]], outs=[warm[:]],
            )

            # layer-3 per-(group,bucket) partial sums [P, g*NCH+k, BC]
            pk = prep.tile([P, NGROUP * NCH, BC], f32, tag="pk")
            nc.vector.memset(pk[:], 0.0)

            def src1(_k):
                return xT[:]

            def src2(k):
                n = SH * P * cumW1[k + 1]
                return gs_[0][:, 0 : n * BC].rearrange("o (r b) -> (o r) b", b=BC)

            def src3(k):
                n = SH * P * cumW2[k + 1]
                return gs_[1][:, 0 : n * BC].rearrange("o (r b) -> (o r) b", b=BC)

            # per layer: (NJ, gdt, tdt, hdt, srcf, idx, w, cins, gs, ship
            #             widths, items, L3 segs)
            layers = [
                (NJ1, bf16, bf16, fp8, src1, idxs[0], wps[0], cins[0], gs_[0],
                 W1SPLIT, bucket_cols([8] * 4), None),
                (NJ2, fp8, bf16, fp8, src2, idxs[1], wps[1], cins[1], gs_[1],
                 W2SPLIT, bucket_cols(w2k, deps=list(range(NCH))), None),
                (NJ3, fp8, bf16, bf16, src3, idxs[2], wps[2], None, None,
                 None, bucket_cols([sum(w) for w in wkg], deps=list(range(NCH))),
                 wkg),
            ]

            qn = [0]  # round-robin SWDGE queue counter

            for li, (NJ, gdt, tdt, hdt, srcf, iad, wp, cin, gout, ships,
                     items, segs) in enumerate(layers):
                last = li == 2
                # ---- coefficient prep: C0..C3 [P, NJ] bf16 ----
                wt = prep.tile([P, NJ * 16], f32, tag="wt")
                nc.sync.dma_start(wt[:], wp[:])
                e = prep.tile([P, NJ * 16], f32, tag="e")
                nc.scalar.activation(e[:], wt[:], Act.Exp)
                e3 = e[:].rearrange("p (j g) -> p j g", g=16)
                e4 = e[:].rearrange("p (j h q) -> p j h q", h=4, q=4)

                ssum = small.tile([P, NJ], f32, tag="ssum")
                nc.vector.reduce_sum(ssum[:], e3, axis=Ax.X)
                r = small.tile([P, NJ], f32, tag="r")
                nc.vector.reciprocal(r[:], ssum[:])

                c0 = small.tile([P, NJ], f32, tag="c0")
                c1 = small.tile([P, NJ], f32, tag="c1")
                c2 = small.tile([P, NJ], f32, tag="c2")
                c3 = small.tile([P, NJ], f32, tag="c3")

                nc.vector.reduce_sum(c0[:], e4[:, :, 2:4, :], axis=Ax.XY)
                t1 = small.tile([P, NJ], f32, tag="t1")
                t2 = small.tile([P, NJ], f32, tag="t2")
                nc.vector.reduce_sum(t1[:], e4[:, :, 0:2, 2:4], axis=Ax.XY)
                nc.vector.reduce_sum(t2[:], e4[:, :, 2:4, 0:2], axis=Ax.XY)
                nc.vector.tensor_sub(c1[:], t1[:], t2[:])
                t3 = small.tile([P, NJ], f32, tag="t3")
                t4 = small.tile([P, NJ], f32, tag="t4")
                nc.vector.reduce_sum(t3[:], e4[:, :, 1, :], axis=Ax.X)
                nc.vector.reduce_sum(t4[:], e4[:, :, 2, :], axis=Ax.X)
                nc.vector.tensor_sub(c2[:], t3[:], t4[:])
                f = small.tile([P, NJ, 7], f32, tag="f")
                nc.vector.tensor_sub(f[:], e3[:, :, 1:8], e3[:, :, 14:7:-1])
                u1 = small.tile([P, NJ], f32, tag="u1")
                u2 = small.tile([P, NJ], f32, tag="u2")
                nc.vector.tensor_sub(u1[:], f[:, :, 0], f[:, :, 1])
                nc.vector.tensor_add(u2[:], f[:, :, 3], f[:, :, 6])
                nc.vector.tensor_sub(u1[:], u1[:], u2[:])
                nc.vector.scalar_tensor_tensor(
                    c3[:], f[:, :, 5], -2.0, u1[:], op0=Alu.mult, op1=Alu.add
                )
                # normalize into bf16 tiles (keeps the combine all-16-bit)
                cb0 = small.tile([P, NJ], bf16, tag="cb0")
                cb1 = small.tile([P, NJ], bf16, tag="cb1")
                cb2 = small.tile([P, NJ], bf16, tag="cb2")
                cb3 = small.tile([P, NJ], bf16, tag="cb3")
                for ck, cbk in zip((c0, c1, c2, c3), (cb0, cb1, cb2, cb3)):
                    nc.vector.tensor_mul(cbk[:], ck[:], r[:])

                # ---- idx load ----
                iab = prep.tile([P, 2 * NJ * 8], i16, tag="iab")
                nc.sync.dma_start(iab[:], iad[:])

                # ---- bucketed gathers + combine ----
                if not last:
                    h = big.tile([P, NJ * BC], hdt, tag="h")
                    h3 = h[:].rearrange("p (j b) -> p j b", b=BC)
                    shipc = cums(ships)
                shipped = 0

                for bi, (j0, j1, dep) in enumerate(items):
                    cw = j1 - j0
                    if cw == 0:
                        continue
                    src = srcf(dep)
                    ab = big.tile([P, 2 * cw, BC], gdt, tag=f"ab{bi}")
                    if last:
                        h3b = big.tile([P, cw, BC], hdt, tag=f"h3{bi % 2}")
                    # a-rows then b-rows; wide streams split across queues.
                    for half, coff in ((0, 0), (1, cw)):
                        nsplit = 2 if cw >= 8 else 1
                        csp = [
                            (cw * t // nsplit, cw * (t + 1) // nsplit)
                            for t in range(nsplit)
                        ]
                        for (s0, s1) in csp:
                            nr = (s1 - s0) * P
                            if nr == 0:
                                continue
                            ioff = 2 * j0 * 8 + half * cw * 8 + s0 * 8
                            nc.gpsimd.dma_gather(
                                ab[:, coff + s0 : coff + s1], src,
                                iab[:, ioff : ioff + (s1 - s0) * 8],
                                nr, nr, BC,
                                single_packet=False, queue_num=qn[0] % 4,
                            )
                            qn[0] += 1
                    tmp = big.tile([P, cw, BC], tdt, tag=f"t{bi}")
                    for j in range(j0, j1):
                        jl = j - j0
                        aj = ab[:, jl]
                        bj = ab[:, cw + jl]
                        # tmp = (a*C3)*b ; tmp = (a*C1)+tmp ; tmp = (b*C2)+tmp
                        nc.vector.scalar_tensor_tensor(
                            tmp[:, jl], aj, cb3[:, j : j + 1], bj,
                            op0=Alu.mult, op1=Alu.mult,
                        )
                        nc.vector.scalar_tensor_tensor(
                            tmp[:, jl], aj, cb1[:, j : j + 1], tmp[:, jl],
                            op0=Alu.mult, op1=Alu.add,
                        )
                        nc.vector.scalar_tensor_tensor(
                            tmp[:, jl], bj, cb2[:, j : j + 1], tmp[:, jl],
                            op0=Alu.mult, op1=Alu.add,
                        )
                        # h = tmp + C0 on the Scalar engine
                        hout = h3b[:, jl] if last else h3[:, j]
                        nc.scalar.activation(
                            hout, tmp[:, jl], Act.Identity,
                            bias=cb0[:, j : j + 1], scale=1.0,
                        )

                    if last:
                        # GroupSum partials for this bucket's segments
                        c = 0
                        for g in range(NGROUP):
                            wseg = segs[bi][g]
                            if wseg == 0:
                                continue
                            sl = h3b[:, c : c + wseg].rearrange(
                                "p j b -> p b j"
                            )
                            nc.vector.reduce_sum(
                                pk[:, g * NCH + bi], sl, axis=Ax.X
                            )
                            c += wseg
                    else:
                        # ship completed AG chunks
                        while shipped < NCH and shipc[shipped + 1] <= j1:
                            m = shipped
                            w0, w1_ = shipc[m], shipc[m + 1]
                            wm = w1_ - w0
                            nc.sync.dma_start(
                                cin[m][:], h[:, w0 * BC : w1_ * BC]
                            )
                            u0 = SH * P * w0
                            un = SH * P * wm
                            nc.gpsimd.collective_compute(
                                "AllGather", Alu.bypass,
                                replica_groups=shard_groups,
                                ins=[cin[m][:]],
                                outs=[gout[:, u0 * BC : (u0 + un) * BC]],
                            )
                            shipped += 1

            # ---- GroupSum: fold buckets, then cross-shard AllGather+sum ----
            gs = prep.tile([P, NGROUP * BC], f32, tag="gs")
            pkv = pk[:].rearrange("p (g k) b -> p g b k", g=NGROUP, k=NCH)
            gsv = gs[:].rearrange("p (g b) -> p g b", b=BC)
            nc.vector.reduce_sum(gsv, pkv, axis=Ax.X)
            ones = prep.tile([P, 1], f32, tag="ones")
            nc.vector.memset(ones[:], 1.0)
            psc = prep.tile([1, NGROUP * BC], f32, tag="psc")
            HW = NGROUP * BC // 2
            for k in range(2):
                ps = psum.tile([1, HW], f32, tag=f"ps{k}")
                nc.tensor.matmul(
                    ps[:], ones[:], gs[:, k * HW : (k + 1) * HW],
                    start=True, stop=True,
                )
                nc.scalar.copy(psc[:, k * HW : (k + 1) * HW], ps[:])
            nc.sync.dma_start(pin[:], psc[:])
            nc.gpsimd.collective_compute(
                "AllGather", Alu.bypass, replica_groups=shard_groups,
                ins=[pin[:]], outs=[pall[:]],
            )
            pall_sb = prep.tile([SH, NGROUP * BC], f32, tag="pall_sb")
            nc.sync.dma_start(pall_sb[:], pall[:])
            ones4 = prep.tile([SH, 1], f32, tag="ones4")
            nc.vector.memset(ones4[:], 1.0)
            osb = prep.tile([1, NGROUP * BC], f32, tag="osb")
            for k in range(2):
                ps2 = psum.tile([1, HW], f32, tag=f"ps2{k}")
                nc.tensor.matmul(
                    ps2[:], ones4[:], pall_sb[:, k * HW : (k + 1) * HW],
                    start=True, stop=True,
                )
                nc.scalar.mul(osb[:, k * HW : (k + 1) * HW], ps2[:], 1.0 / TAU)
            # consume the warm-up collective's (all-zero) output so DCE keeps it
            wsb2 = prep.tile([1, 16], f32, tag="wsb2")
            nc.sync.dma_start(wsb2[:], warm[0:1, :])
            nc.vector.tensor_add(osb[:, :16], osb[:, :16], wsb2[:])
            nc.sync.dma_start(out_d[:], osb[:])

    nc.compile()
    return nc


def _wrap_idx(ii):
    w = ii.astype(np.int16).reshape(-1, 16).T
    return np.ascontiguousarray(np.tile(w, (8, 1)))


def _pack_w(w_eff, nj):
    # local slot t = j*128 + p  ->  packed[p, j*16+g]
    return np.ascontiguousarray(
        w_eff.reshape(nj, P, 16).transpose(1, 0, 2).reshape(P, nj * 16)
    )


def _cums(ws):
    out = [0]
    for w in ws:
        out.append(out[-1] + w)
    return out


def _chunk_of_col(j, cum):
    """AG chunk index of column j given cumulative chunk boundaries."""
    return np.searchsorted(np.asarray(cum[1:]), j, side="right")


def _unit_of(s, j, p, cum):
    """Flat gs unit for shard s, col j, partition p with chunk widths cum."""
    k = _chunk_of_col(j, cum)
    c0 = np.asarray(cum)[k]
    w = np.asarray(cum)[k + 1] - c0
    return SH * P * c0 + s * (P * w) + p * w + (j - c0)


def _host_pack(inputs):
    x = np.asarray(inputs["x"], dtype=np.float32)
    w1 = np.asarray(inputs["w1"], dtype=np.float32)
    w2 = np.asarray(inputs["w2"], dtype=np.float32)
    w3 = np.asarray(inputs["w3"], dtype=np.float32)
    i1a = np.asarray(inputs["idx1a"]).astype(np.int64)
    i1b = np.asarray(inputs["idx1b"]).astype(np.int64)
    i2a = np.asarray(inputs["idx2a"]).astype(np.int64)
    i2b = np.asarray(inputs["idx2b"]).astype(np.int64)
    i3a = np.asarray(inputs["idx3a"]).astype(np.int64)
    i3b = np.asarray(inputs["idx3b"]).astype(np.int64)

    pad_row = np.full(16, -20.0, dtype=np.float32)
    pad_row[0] = 20.0  # softmax -> ~one-hot FALSE gate -> h = 0

    cum1 = _cums(W1SPLIT)
    per_shard = [dict() for _ in range(SH)]

    # ---- layer 1: shard s owns neurons [s*4000, (s+1)*4000), slot = rank;
    # 4 fixed 8-col gather items; idx stream = [a-cols | b-cols] per item.
    NS1 = NJ1 * P
    for s in range(SH):
        sel = slice(s * REAL12, (s + 1) * REAL12)
        w_eff = np.concatenate(
            [w1[sel], np.tile(pad_row, (NS1 - REAL12, 1))], axis=0
        )
        ia_eff = np.zeros(NS1, dtype=np.int64)
        ib_eff = np.zeros(NS1, dtype=np.int64)
        ia_eff[:REAL12] = i1a[sel]
        ib_eff[:REAL12] = i1b[sel]
        parts = []
        for k in range(4):
            lo, hi = k * 8 * P, (k + 1) * 8 * P
            parts.append(ia_eff[lo:hi])
            parts.append(ib_eff[lo:hi])
        per_shard[s]["w1p"] = _pack_w(w_eff, NJ1)
        per_shard[s]["i1"] = _wrap_idx(np.concatenate(parts))

    # layer-1 neuron id -> (shard, col, p) -> AG1 chunk + gs_[0] unit
    def loc1(i):
        s = i // REAL12
        t = i - s * REAL12
        return s, t // P, t % P

    def chunk1(i):
        _, j, _ = loc1(i)
        return _chunk_of_col(j, cum1)

    def unit1(i):
        s, j, p = loc1(i)
        return _unit_of(s, j, p, cum1)

    # ---- layer 2: bucket-sort each shard's neurons by max source AG1 chunk.
    ord2, cnt2 = [], np.zeros((SH, NCH), dtype=np.int64)
    for s in range(SH):
        sel = np.arange(s * REAL12, (s + 1) * REAL12)
        m = np.maximum(chunk1(i2a[sel]), chunk1(i2b[sel]))
        o = np.argsort(m, kind="stable")
        ord2.append(sel[o])
        cnt2[s] = np.bincount(m, minlength=NCH)
    w2k = [int(np.ceil(cnt2[:, k].max() / P)) for k in range(NCH)]
    nj2 = sum(w2k)
    from_split = _sqrt_split(nj2)
    w2split = from_split
    cum2 = _cums(w2split)

    # slot tables: layer-2 neuron id -> (shard, col, p)
    s2t = np.full(SH * REAL12, -1, dtype=np.int64)
    j2t = np.full(SH * REAL12, -1, dtype=np.int64)
    p2t = np.full(SH * REAL12, -1, dtype=np.int64)
    cumb2 = _cums(w2k)
    for s in range(SH):
        ids = ord2[s]
        pos = 0
        for k in range(NCH):
            n = int(cnt2[s][k])
            bsl = ids[pos : pos + n]
            slot = cumb2[k] * P + np.arange(n)
            s2t[bsl] = s
            j2t[bsl] = slot // P
            p2t[bsl] = slot % P
            pos += n

    def chunk2(i):
        return _chunk_of_col(j2t[i], cum2)

    def unit2(i):
        return _unit_of(s2t[i], j2t[i], p2t[i], cum2)

    NS2 = nj2 * P
    for s in range(SH):
        ids = ord2[s]
        w_eff = np.tile(pad_row, (NS2, 1))
        ia_eff = np.zeros(NS2, dtype=np.int64)
        ib_eff = np.zeros(NS2, dtype=np.int64)
        parts = []
        pos = 0
        for k in range(NCH):
            n = int(cnt2[s][k])
            wk = w2k[k]
            lo = cumb2[k] * P
            bsl = ids[pos : pos + n]
            w_eff[lo : lo + n] = w2[bsl]
            ia_eff[lo : lo + n] = unit1(i2a[bsl])
            ib_eff[lo : lo + n] = unit1(i2b[bsl])
            parts.append(ia_eff[lo : lo + wk * P])
            parts.append(ib_eff[lo : lo + wk * P])
            pos += n
        per_shard[s]["w2p"] = _pack_w(w_eff, nj2)
        per_shard[s]["i2"] = _wrap_idx(np.concatenate(parts))

    # ---- layer 3: shard split per group, bucket by max AG2 chunk,
    # segmented by (bucket, group).
    counts3 = np.array([1334, 1333, 1333, 1333])
    offs3 = np.concatenate([[0], np.cumsum(counts3)[:-1]])
    ids3 = []
    cnt3 = np.zeros((SH, NCH, NGROUP), dtype=np.int64)
    for s in range(SH):
        d = {}
        for g in range(NGROUP):
            rid = g * SPG + offs3[s] + np.arange(counts3[s])
            m = np.maximum(chunk2(i3a[rid]), chunk2(i3b[rid]))
            for k in range(NCH):
                d[(k, g)] = rid[m == k]
                cnt3[s, k, g] = len(d[(k, g)])
        ids3.append(d)
    wkg = [
        [int(np.ceil(cnt3[:, k, g].max() / P)) for g in range(NGROUP)]
        for k in range(NCH)
    ]
    nj3 = sum(sum(w) for w in wkg)

    NS3 = nj3 * P
    for s in range(SH):
        w_eff = np.tile(pad_row, (NS3, 1))
        ia_eff = np.zeros(NS3, dtype=np.int64)
        ib_eff = np.zeros(NS3, dtype=np.int64)
        parts = []
        scol = 0
        for k in range(NCH):
            bcol0 = scol
            for g in range(NGROUP):
                rid = ids3[s][(k, g)]
                n = len(rid)
                lo = scol * P
                if n:
                    w_eff[lo : lo + n] = w3[rid]
                    ia_eff[lo : lo + n] = unit2(i3a[rid])
                    ib_eff[lo : lo + n] = unit2(i3b[rid])
                scol += wkg[k][g]
            blo, bhi = bcol0 * P, scol * P
            parts.append(ia_eff[blo:bhi])
            parts.append(ib_eff[blo:bhi])
        per_shard[s]["w3p"] = _pack_w(w_eff, nj3)
        per_shard[s]["i3"] = _wrap_idx(np.concatenate(parts))

    shapes = dict(w2k=w2k, nj2=nj2, w2split=w2split, wkg=wkg, nj3=nj3)

    import ml_dtypes

    in_maps = []
    for c in range(N_CORES):
        G, s = c // SH, c % SH
        m_ = dict(per_shard[s])
        m_["xT"] = np.ascontiguousarray(
            x[G * BC : (G + 1) * BC].T.astype(ml_dtypes.bfloat16)
        )
        in_maps.append(m_)
    return in_maps, shapes


LAST_RESULTS = None


def kernel(**inputs):
    global LAST_RESULTS
    from concourse.bass_utils import run_bass_kernel_spmd

    in_maps, shapes = _host_pack(inputs)
    if "nc" not in _CACHE:
        _CACHE["nc"] = _build_nc(shapes)
    nc = _CACHE["nc"]

    trace = bool(int(os.environ.get("KERNEL_TRACE", "0")))
    res = run_bass_kernel_spmd(
        nc, in_maps, core_ids=list(range(N_CORES)), trace=trace
    )
    LAST_RESULTS = res

    out = np.empty((B, NGROUP), dtype=np.float32)
    for g_ in range(BG):
        rc = res.results[g_ * SH]["out"].reshape(NGROUP, BC)
        out[g_ * BC : (g_ + 1) * BC, :] = rc.T
    return out
